# revision 15
# baseline (speedup 1.0000x reference)
"""Trainium2 Bass kernel for nn_DeepStreamOutput (nms_detection).

Sharding: data-parallel over batch (2) x detection-quarter (4) = 8 cores.
Each core gets its batch element's full inputs, computes top-100 selection
(exact, stable ties by flat index), then RoIAlign+mask-matmul for ITS 25
detections, writing a [25, 25606] block. Host assembles [2, 100, 25606].

Pipeline per core:
  1. gpsimd.topk per 61440-token (11 tokens, 2 instrs) -> top-32/token
     candidates (352) with indices.
  2. Exact global rank of each candidate among candidates by
     (value desc, flat asc) via fused DVE compare-accumulate ops.
  3. Candidates with rank in [off, off+25) scatter (sigmoid(val), flat) to
     a DRAM scratch row (rank - off); readback gives the sorted 25.
  4. Gather box-idx (LUT), boxes row, masks row per det; decode
     cxcywh->xyxy (proto coords); build sampling grids.
  5. mask logits M = masksel @ protos via PE ([25,480] PSUM chunks).
  6. Separable bilinear RoIAlign as two per-det PE matmuls against
     hat-function interp matrices built on ACT/DVE:
       tmpT = M_t^T @ GyhatT ; mp = tmpT^T @ GxhatT
  7. mask_bias added during final PSUM eviction; DMA out.
"""
import sys
sys.path.insert(0, "/opt/trn_rl_repo")
sys.path.insert(0, "/root/.axon_site/_ro/trn_rl_repo")
import numpy as np
from contextlib import ExitStack

import concourse.bass as bass
import concourse.tile as tile
from concourse import bacc, mybir, library_config
from concourse.bass import IndirectOffsetOnAxis
from concourse.bass_utils import run_bass_kernel_spmd

F32 = mybir.dt.float32
U32 = mybir.dt.uint32
I32 = mybir.dt.int32

VOCAB = 61440          # padded per-token vocab (3840 per partition row)
NEG = -1.0e30
CAP = 384              # rank candidate capacity (352 real + pads)
NCAND_P = 22           # candidate partitions (2 per token x 11 tokens)
MD = 25                # dets per core
OUTW = 25606


def emit_kernel(tc, ins, outs):
    nc = tc.nc
    ctx = ExitStack()
    A = mybir.AluOpType

    # ---------------- static sbuf tensors (ISA ops need real handles) ----
    sc1 = nc.alloc_sbuf_tensor("sb_sc1", [128, 3840], F32).ap()
    sc2 = nc.alloc_sbuf_tensor("sb_sc2", [48, 3840], F32).ap()
    tk1 = nc.alloc_sbuf_tensor("sb_tk1", [128, 32], U32).ap()
    tk2 = nc.alloc_sbuf_tensor("sb_tk2", [48, 32], U32).ap()
    vflat = nc.alloc_sbuf_tensor("sb_vflat", [1, CAP], F32).ap()
    fflat = nc.alloc_sbuf_tensor("sb_fflat", [1, CAP], F32).ap()
    vcol = nc.alloc_sbuf_tensor("sb_vcol", [128, 3], F32).ap()
    fcol = nc.alloc_sbuf_tensor("sb_fcol", [128, 3], F32).ap()
    rank = nc.alloc_sbuf_tensor("sb_rank", [128, 3], F32).ap()
    acc = nc.alloc_sbuf_tensor("sb_acc", [128, CAP], F32).ap()
    sigc = nc.alloc_sbuf_tensor("sb_sigc", [128, 3], F32).ap()
    packed = nc.alloc_sbuf_tensor("sb_packed", [128, 3, 2], F32).ap()
    sb25 = nc.alloc_sbuf_tensor("sb_sb25", [MD, 2], F32).ap()
    flatu = nc.alloc_sbuf_tensor("sb_flatu", [MD, 1], U32).ap()
    boxu = nc.alloc_sbuf_tensor("sb_boxu", [MD, 1], U32).ap()
    cxy = nc.alloc_sbuf_tensor("sb_cxy", [MD, 4], F32).ap()
    mrow = nc.alloc_sbuf_tensor("sb_mrow", [MD, 32], F32).ap()

    # pools for big transients
    pool = ctx.enter_context(tc.tile_pool(name="big", bufs=1))
    ppool = ctx.enter_context(tc.tile_pool(name="pr", bufs=2))
    psum = ctx.enter_context(
        tc.tile_pool(name="ps", bufs=4, space=bass.MemorySpace.PSUM))
    psum2 = ctx.enter_context(
        tc.tile_pool(name="ps2", bufs=1, space=bass.MemorySpace.PSUM))

    # stage small consts into SBUF
    const_shapes = {"tokbase": (NCAND_P, 1), "ones": (1, 128),
                    "ident": (128, 128), "offcol": (128, 1),
                    "biascol": (128, 1), "hgrid": (MD, 160), "yneg": (80, 1),
                    "selbc": (MD, MD * 80), "slotgrid": (128, 27)}
    cst = {}
    for name, shp in const_shapes.items():
        t = nc.alloc_sbuf_tensor("sbc_" + name, list(shp), F32).ap()
        nc.sync.dma_start(t, ins[name])
        cst[name] = t
    ins = {**ins, **cst}

    scores = ins["scores_b"]
    # ------------- 1. topk per token ------------------------------------
    nc.vector.memset(sc2[:, :], NEG)
    nc.sync.dma_start(sc1[:, :],
                      scores[0:128 * 3840].rearrange("(p f) -> p f", p=128))
    nc.sync.dma_start(sc2[0:32, :],
                      scores[8 * VOCAB:8 * VOCAB + 32 * 3840]
                      .rearrange("(p f) -> p f", p=32))
    nc.sync.dma_start(sc2[32:47, :],
                      scores[10 * VOCAB:10 * VOCAB + 15 * 3840]
                      .rearrange("(p f) -> p f", p=15))
    with tc.tile_critical():
        nc.gpsimd.load_library(library_config.topk)
        nc.gpsimd.topk(tk1[:], sc1[:], tokens=8, vocab_size=VOCAB, k=256)
        nc.gpsimd.topk(tk2[:], sc2[:], tokens=3, vocab_size=VOCAB, k=256)

    # ------------- 2. candidates -> [1, CAP] rows -----------------------
    # top-32 of token t = partitions 16t+14, 16t+15 (ascending vals cols
    # 0..15, idx cols 16..31)
    nc.vector.memset(vflat, NEG)
    nc.vector.memset(fflat, -1.0)
    cv = nc.alloc_sbuf_tensor("sb_cv", [NCAND_P, 16], F32).ap()
    cfi = nc.alloc_sbuf_tensor("sb_cfi", [NCAND_P, 16], U32).ap()
    cff = nc.alloc_sbuf_tensor("sb_cff", [NCAND_P, 16], F32).ap()
    for t in range(11):
        srcv = tk1 if t < 8 else tk2
        pb = 16 * (t if t < 8 else t - 8)
        nc.sync.dma_start(cv[2 * t:2 * t + 2, :],
                          srcv[pb + 14:pb + 16, 0:16].bitcast(F32))
        nc.sync.dma_start(cfi[2 * t:2 * t + 2, :],
                          srcv[pb + 14:pb + 16, 16:32])
    nc.vector.tensor_copy(cff, cfi)  # exact ints
    nc.vector.tensor_scalar(cff, cff, ins_tokbase(nc, ins), None, op0=A.add)
    # resort to [1, 352]: r = p*16 + s
    nc.sync.dma_start(vflat[:, 0:NCAND_P * 16], cv)
    nc.sync.dma_start(fflat[:, 0:NCAND_P * 16], cff)
    # cols: vcol[p,c] = vflat[3p+c] (lockstep DMA iteration)
    nc.sync.dma_start(vcol, vflat)
    nc.sync.dma_start(fcol, fflat)
    # V_row/F_row = ones-matmul broadcasts (PSUM)
    vrow = psum2.tile([128, CAP], F32)
    frow = psum2.tile([128, CAP], F32)
    nc.tensor.matmul(vrow[:], ins["ones"], vflat, start=True, stop=True)
    nc.tensor.matmul(frow[:], ins["ones"], fflat, start=True, stop=True)
    # ------------- ranks ------------------------------------------------
    for c in range(3):
        nc.vector.tensor_scalar(acc, vrow[:], vcol[:, c:c + 1], None,
                                op0=A.is_equal)
        nc.vector.scalar_tensor_tensor(acc, frow[:], fcol[:, c:c + 1], acc,
                                       op0=A.is_lt, op1=A.mult)
        nc.vector.scalar_tensor_tensor(acc, vrow[:], vcol[:, c:c + 1], acc,
                                       op0=A.is_gt, op1=A.add,
                                       accum_out=rank[:, c:c + 1])
    # ------------- targets + scatter ------------------------------------
    tgtf = nc.alloc_sbuf_tensor("sb_tgtf", [128, 3], F32).ap()
    nc.vector.tensor_scalar(tgtf, rank, ins["offcol"], None, op0=A.subtract)
    neg26 = nc.alloc_sbuf_tensor("sb_neg26", [128, 3], F32).ap()
    nc.vector.memset(neg26, 26.0)
    ltz = nc.alloc_sbuf_tensor("sb_ltz", [128, 3], mybir.dt.uint8).ap()
    lzf = nc.alloc_sbuf_tensor("sb_lzf", [128, 3], F32).ap()
    nc.vector.tensor_scalar(lzf, tgtf, 0.0, None, op0=A.is_lt)
    nc.vector.tensor_copy(ltz, lzf)
    nc.vector.tensor_scalar(tgtf, tgtf, 26.0, None, op0=A.min)
    nc.vector.copy_predicated(tgtf, ltz, neg26)
    nc.scalar.activation(sigc, vcol, mybir.ActivationFunctionType.Sigmoid)
    # pack [sig, flat]
    nc.vector.tensor_copy(packed[:, :, 0:1], sigc.rearrange("p (c x) -> p c x", x=1))
    nc.vector.tensor_copy(packed[:, :, 1:2], fcol.rearrange("p (c x) -> p c x", x=1))
    # de-scatter via permutation matmul: sorted[slot,:] = sum_r
    # 1[tgt_r == slot] * packed[r,:]   (slot grid is a host const [128,27])
    pmat = nc.alloc_sbuf_tensor("sb_pmat", [128, 27], F32).ap()
    srt_ps = psum.tile([27, 2], F32, tag="w")
    for c in range(3):
        nc.vector.tensor_scalar(pmat, ins["slotgrid"], tgtf[:, c:c + 1], None,
                                op0=A.is_equal)
        nc.tensor.matmul(srt_ps[:], pmat, packed[:, c, :],
                         start=(c == 0), stop=(c == 2))
    nc.vector.tensor_copy(sb25, srt_ps[0:MD, :])
    nc.vector.tensor_copy(flatu, sb25[:, 1:2])
    nc.gpsimd.indirect_dma_start(
        out=boxu,
        out_offset=None,
        in_=ins["boxlut"],
        in_offset=IndirectOffsetOnAxis(ap=flatu, axis=0),
        bounds_check=671999, oob_is_err=False)
    nc.gpsimd.indirect_dma_start(
        out=cxy,
        out_offset=None, in_=ins["boxes_b"],
        in_offset=IndirectOffsetOnAxis(ap=boxu, axis=0),
        bounds_check=8399, oob_is_err=False)
    nc.gpsimd.indirect_dma_start(
        out=mrow,
        out_offset=None, in_=ins["masks_b"],
        in_offset=IndirectOffsetOnAxis(ap=boxu, axis=0),
        bounds_check=8399, oob_is_err=False)
    # label = flat - 80*box (int)
    lab_i = nc.alloc_sbuf_tensor("sb_lab_i", [MD, 1], I32).ap()
    box_i = nc.alloc_sbuf_tensor("sb_box_i", [MD, 1], I32).ap()
    fl_i = nc.alloc_sbuf_tensor("sb_fl_i", [MD, 1], I32).ap()
    nc.vector.tensor_copy(box_i, boxu)
    nc.vector.tensor_copy(fl_i, flatu)
    nc.vector.tensor_scalar(box_i, box_i, 80, None, op0=A.mult)
    nc.vector.tensor_tensor(lab_i, fl_i, box_i, op=A.subtract)
    labf = nc.alloc_sbuf_tensor("sb_labf", [MD, 1], F32).ap()
    nc.vector.tensor_copy(labf, lab_i)
    # masksel transpose [25,32] -> [32,25]
    mT_ps = psum.tile([32, MD], F32, tag="w")
    nc.tensor.transpose(out=mT_ps[:], in_=mrow, identity=ins["ident"][0:MD, 0:MD])
    mT = nc.alloc_sbuf_tensor("sb_mT", [32, MD], F32).ap()
    nc.vector.tensor_copy(mT, mT_ps[:])
    # ------------- geometry ---------------------------------------------
    # proto coords: x1p=(cx-w/2)*160 etc; bins: binw=(x2p-x1p)/160 = w*1.0
    g = nc.alloc_sbuf_tensor("sb_g", [MD, 8], F32).ap()
    cx, cy, w, h = (cxy[:, i:i + 1] for i in range(4))
    x1p, y1p, x2p, y2p = (g[:, i:i + 1] for i in range(4))
    binw, binh = g[:, 4:5], g[:, 5:6]
    nc.vector.scalar_tensor_tensor(x1p, w, -0.5, cx, op0=A.mult, op1=A.add)
    nc.vector.scalar_tensor_tensor(y1p, h, -0.5, cy, op0=A.mult, op1=A.add)
    nc.vector.scalar_tensor_tensor(x2p, w, 0.5, cx, op0=A.mult, op1=A.add)
    nc.vector.scalar_tensor_tensor(y2p, h, 0.5, cy, op0=A.mult, op1=A.add)
    for col in (x1p, y1p, x2p, y2p):
        nc.vector.tensor_scalar(col, col, 160.0, None, op0=A.mult)
    nc.vector.tensor_scalar(binw, w, 1.0, None, op0=A.mult)  # w*640*0.25/160
    nc.vector.tensor_scalar(binh, h, 1.0, None, op0=A.mult)
    # grids ys2 [25, 320] = [clip(ys) || clip(ys)-80], xs2 likewise
    ys2 = nc.alloc_sbuf_tensor("sb_ys2", [MD, 320], F32).ap()
    xs2 = nc.alloc_sbuf_tensor("sb_xs2", [MD, 320], F32).ap()
    for gi, (dst, b0, c0) in enumerate(((ys2, binh, y1p), (xs2, binw, x1p))):
        off05 = nc.alloc_sbuf_tensor(f"sb_off05_{gi}", [MD, 1], F32).ap()
        nc.vector.tensor_scalar(off05, c0, 0.5, None, op0=A.subtract)
        nc.vector.tensor_scalar(dst[:, 0:160], ins["hgrid"], b0, off05,
                                op0=A.mult, op1=A.add)
        nc.vector.tensor_scalar(dst[:, 0:160], dst[:, 0:160], 0.0, 159.0,
                                op0=A.max, op1=A.min)
        nc.vector.tensor_scalar(dst[:, 160:320], dst[:, 0:160], 80.0, None,
                                op0=A.subtract)
    # rows [1, 8000] (order t, yc, h)

    # ------------- hat matrices  [80, 8000] -----------------------------
    # hatY[y', (t,yc,h)] = -(min(|ys2row - y'|,1) - 1) sign-pair cancels
    def build_hat(hat, row):
        for ch in range(MD):
            sl = slice(320 * ch, 320 * (ch + 1))
            bc = psum.tile([80, 320], F32, tag="w")
            nc.tensor.matmul(bc[:], ins["selbc"][:, 80 * ch:80 * ch + 80],
                             row, start=True, stop=True)
            nc.scalar.activation(hat[:, sl], bc[:],
                                 mybir.ActivationFunctionType.Abs,
                                 bias=ins["yneg"])
        # pass2: -hat = min(a,1) - 1  (fused)
        for half in range(2):
            sl = slice(4000 * half, 4000 * (half + 1))
            nc.vector.tensor_scalar(hat[:, sl], hat[:, sl], 1.0, 1.0,
                                    op0=A.min, op1=A.subtract)

    hatY = pool.tile([80, 8000], F32, tag="hat")
    build_hat(hatY, ys2)
    # ------------- M matmul + eviction + reshape ------------------------
    mdram = nc.dram_tensor("mdram", [MD, 160, 160], F32).ap()
    mdflat = mdram.rearrange("t y x -> t (y x)")
    for c in range(54):
        n = 480 if c < 53 else 160
        pch = ppool.tile([32, 480], F32, tag="pch")
        nc.sync.dma_start(pch[:, 0:n], ins["protos_b"][:, 480 * c:480 * c + n])
        mp_ps = psum.tile([MD, 480], F32, tag="w")
        nc.tensor.matmul(mp_ps[:, 0:n], mT, pch[:, 0:n], start=True, stop=True)
        mchunk = ppool.tile([MD, 480], F32, tag="mchunk")
        if c % 2 == 0:
            nc.vector.tensor_copy(mchunk[:, 0:n], mp_ps[:, 0:n])
        else:
            nc.scalar.copy(mchunk[:, 0:n], mp_ps[:, 0:n])
        nc.sync.dma_start(mdflat[:, 480 * c:480 * c + n], mchunk[:, 0:n])
    mxy = pool.tile([80, MD, 2, 160], F32, tag="mxy")
    nc.sync.dma_start(
        mxy[:], mdram.rearrange("t (yc y) x -> y t yc x", y=80))
    # ------------- stage A: tmpT = M^T @ GyhatT -------------------------
    tx = pool.tile([80, MD, 2, 160], F32, tag="tx")
    for t in range(MD):
        for xc in range(2):
            pa = psum.tile([80, 160], F32, tag="w")
            for yc in range(2):
                nc.tensor.matmul(pa[:], mxy[:, t, yc, 80 * xc:80 * xc + 80],
                                 hatY[:, (t * 2 + yc) * 160:(t * 2 + yc) * 160 + 160],
                                 start=(yc == 0), stop=(yc == 1))
            if (t + xc) % 2 == 0:
                nc.vector.tensor_copy(tx[:, t, xc, :], pa[:])
            else:
                nc.scalar.copy(tx[:, t, xc, :], pa[:])
    # ------------- stage B: mp = tmpT^T @ GxhatT + bias -----------------
    hatX = pool.tile([80, 8000], F32, tag="hat")
    build_hat(hatX, xs2)
    mph0 = pool.tile([128, MD, 160], F32, tag="mph0")
    mph1 = pool.tile([32, MD, 160], F32, tag="mph1")
    for t in range(MD):
        for hc in range(2):
            hn = 128 if hc == 0 else 32
            pb = psum.tile([hn, 160], F32, tag="w")
            for xc in range(2):
                nc.tensor.matmul(pb[:], tx[:, t, xc, 128 * hc:128 * hc + hn],
                                 hatX[:, (t * 2 + xc) * 160:(t * 2 + xc) * 160 + 160],
                                 start=(xc == 0), stop=(xc == 1))
            dst = mph0[:, t, :] if hc == 0 else mph1[:, t, :]
            nc.scalar.activation(dst, pb[:],
                                 mybir.ActivationFunctionType.Identity,
                                 bias=ins["biascol"][0:hn, :])
    # ------------- output writes ----------------------------------------
    ob = outs["out_block"]
    nc.sync.dma_start(
        ob[:, 6:6 + 128 * 160].rearrange("t (y x) -> y t x", y=128), mph0[:])
    nc.sync.dma_start(
        ob[:, 6 + 128 * 160:6 + 160 * 160].rearrange("t (y x) -> y t x", y=32),
        mph1[:])
    if "dbg_vflat" in outs:
        nc.sync.dma_start(outs["dbg_vflat"], vflat)
        nc.sync.dma_start(outs["dbg_fflat"], fflat)
        nc.sync.dma_start(outs["dbg_rank"], rank)
        nc.sync.dma_start(outs["dbg_sb25"], sb25)
        nc.sync.dma_start(outs["dbg_cxy"], cxy)
        nc.sync.dma_start(outs["dbg_vcol"], vcol)
        nc.sync.dma_start(outs["dbg_fcol"], fcol)
        nc.sync.dma_start(outs["dbg_tgt"], tgtf)
        nc.sync.dma_start(outs["dbg_packed"], packed.rearrange("p a b -> p (a b)"))
    asm = nc.alloc_sbuf_tensor("sb_asm", [MD, 6], F32).ap()
    for i, col in enumerate((x1p, y1p, x2p, y2p)):
        nc.vector.tensor_scalar(asm[:, i:i + 1], col, 4.0, None, op0=A.mult)
    nc.vector.tensor_copy(asm[:, 4:5], sb25[:, 0:1])
    nc.vector.tensor_copy(asm[:, 5:6], labf)
    nc.sync.dma_start(ob[:, 0:6], asm)
    ctx.close()


def ins_tokbase(nc, ins):
    return ins["tokbase"]


_BUILD_CACHE = {}


def build():
    if "nc" in _BUILD_CACHE:
        return _BUILD_CACHE["nc"]
    nc = bacc.Bacc("TRN2", target_bir_lowering=False, debug=False, num_devices=8)
    ins, outs = {}, {}

    def inp(name, shape, dt):
        ins[name] = nc.dram_tensor(name, list(shape), dt, kind="ExternalInput").ap()

    inp("scores_b", (672000,), F32)
    inp("protos_b", (32, 25600), F32)
    inp("masks_b", (8400, 32), F32)
    inp("boxes_b", (8400, 4), F32)
    inp("boxlut", (672000, 1), U32)
    inp("tokbase", (NCAND_P, 1), F32)
    inp("ones", (1, 128), F32)
    inp("ident", (128, 128), F32)
    inp("offcol", (128, 1), F32)
    inp("biascol", (128, 1), F32)
    inp("hgrid", (MD, 160), F32)
    inp("yneg", (80, 1), F32)
    inp("selbc", (MD, MD * 80), F32)
    inp("slotgrid", (128, 27), F32)
    outs["out_block"] = nc.dram_tensor("out_block", [MD, OUTW], F32,
                                       kind="ExternalOutput").ap()
    import os
    if os.environ.get("KDBG") == "1":
        for nm, shp in (("dbg_vflat", (1, CAP)), ("dbg_fflat", (1, CAP)),
                        ("dbg_rank", (128, 3)), ("dbg_sb25", (MD, 2)),
                        ("dbg_cxy", (MD, 4)), ("dbg_vcol", (128, 3)),
                        ("dbg_fcol", (128, 3)), ("dbg_tgt", (128, 3)),
                        ("dbg_packed", (128, 6))):
            outs[nm] = nc.dram_tensor(nm, list(shp), F32,
                                      kind="ExternalOutput").ap()
    with tile.TileContext(nc) as tc:
        emit_kernel(tc, ins, outs)
    nc.compile()
    _BUILD_CACHE["nc"] = nc
    return nc


def make_in_maps(boxes, scores, protos, masks, mask_bias):
    B = scores.shape[0]
    boxlut = (np.arange(672000, dtype=np.uint32) // 80).astype(np.uint32)[:, None]
    tokbase = np.repeat(np.arange(11, dtype=np.float32) * VOCAB, 2)[:, None]
    ones = np.ones((1, 128), np.float32)
    ident = np.eye(128, dtype=np.float32)
    hgrid = np.tile(np.arange(160, dtype=np.float32) + 0.5, (MD, 1))
    yneg = -np.arange(80, dtype=np.float32)[:, None]
    selbc = np.zeros((MD, MD * 80), np.float32)
    for t in range(MD):
        selbc[t, 80 * t:80 * t + 80] = 1.0
    in_maps = []
    for core in range(8):
        b, q = core // 4, core % 4
        in_maps.append({
            "scores_b": np.ascontiguousarray(scores[b].reshape(-1)),
            "protos_b": np.ascontiguousarray(protos[b].reshape(32, 25600)),
            "masks_b": np.ascontiguousarray(masks[b]),
            "boxes_b": np.ascontiguousarray(boxes[b]),
            "boxlut": boxlut,
            "tokbase": tokbase.astype(np.float32),
            "ones": ones, "ident": ident,
            "offcol": np.full((128, 1), 25.0 * q, np.float32),
            "biascol": np.full((128, 1), float(mask_bias[0]), np.float32),
            "hgrid": hgrid, "yneg": yneg, "selbc": selbc,
            "slotgrid": np.tile(np.arange(27, dtype=np.float32), (128, 1)),
        })
    return in_maps


def kernel(boxes, scores, protos, masks, mask_bias):
    nc = build()
    in_maps = make_in_maps(boxes, scores, protos, masks, mask_bias)
    res = run_bass_kernel_spmd(nc, in_maps, list(range(8)))
    out = np.zeros((2, 100, OUTW), np.float32)
    for core in range(8):
        b, q = core // 4, core % 4
        out[b, 25 * q:25 * q + 25] = res.results[core]["out_block"]
    return out


# revision 17
# speedup vs baseline: 1.0106x; 1.0106x over previous
"""Trainium2 Bass kernel for nn_DeepStreamOutput (nms_detection).

Sharding: data-parallel over batch (2) x detection-quarter (4) = 8 cores.
Each core gets its batch element's full inputs, computes top-100 selection
(exact, stable ties by flat index), then RoIAlign+mask-matmul for ITS 25
detections, writing a [25, 25606] block. Host assembles [2, 100, 25606].

Pipeline per core:
  1. gpsimd.topk per 61440-token (11 tokens, 2 instrs) -> top-32/token
     candidates (352) with indices.
  2. Exact global rank of each candidate among candidates by
     (value desc, flat asc) via fused DVE compare-accumulate ops.
  3. Candidates with rank in [off, off+25) scatter (sigmoid(val), flat) to
     a DRAM scratch row (rank - off); readback gives the sorted 25.
  4. Gather box-idx (LUT), boxes row, masks row per det; decode
     cxcywh->xyxy (proto coords); build sampling grids.
  5. mask logits M = masksel @ protos via PE ([25,480] PSUM chunks).
  6. Separable bilinear RoIAlign as two per-det PE matmuls against
     hat-function interp matrices built on ACT/DVE:
       tmpT = M_t^T @ GyhatT ; mp = tmpT^T @ GxhatT
  7. mask_bias added during final PSUM eviction; DMA out.
"""
import sys
sys.path.insert(0, "/opt/trn_rl_repo")
sys.path.insert(0, "/root/.axon_site/_ro/trn_rl_repo")
import numpy as np
from contextlib import ExitStack

import concourse.bass as bass
import concourse.tile as tile
from concourse import bacc, mybir, library_config
from concourse.bass import IndirectOffsetOnAxis
from concourse.bass_utils import run_bass_kernel_spmd

F32 = mybir.dt.float32
U32 = mybir.dt.uint32
I32 = mybir.dt.int32

VOCAB = 61440          # padded per-token vocab (3840 per partition row)
NEG = -1.0e30
CAP = 384              # rank candidate capacity (352 real + pads)
NCAND_P = 22           # candidate partitions (2 per token x 11 tokens)
MD = 25                # dets per core
OUTW = 25606


def emit_kernel(tc, ins, outs):
    nc = tc.nc
    ctx = ExitStack()
    A = mybir.AluOpType

    # ---------------- static sbuf tensors (ISA ops need real handles) ----
    sc1 = nc.alloc_sbuf_tensor("sb_sc1", [128, 3840], F32).ap()
    sc2 = nc.alloc_sbuf_tensor("sb_sc2", [48, 3840], F32).ap()
    tk1 = nc.alloc_sbuf_tensor("sb_tk1", [128, 32], U32).ap()
    tk2 = nc.alloc_sbuf_tensor("sb_tk2", [48, 32], U32).ap()
    vflat = nc.alloc_sbuf_tensor("sb_vflat", [1, CAP], F32).ap()
    fflat = nc.alloc_sbuf_tensor("sb_fflat", [1, CAP], F32).ap()
    vcol = nc.alloc_sbuf_tensor("sb_vcol", [128, 3], F32).ap()
    fcol = nc.alloc_sbuf_tensor("sb_fcol", [128, 3], F32).ap()
    rank = nc.alloc_sbuf_tensor("sb_rank", [128, 3], F32).ap()
    acc = nc.alloc_sbuf_tensor("sb_acc", [128, CAP], F32).ap()
    sigc = nc.alloc_sbuf_tensor("sb_sigc", [128, 3], F32).ap()
    packed = nc.alloc_sbuf_tensor("sb_packed", [128, 3, 2], F32).ap()
    sb25 = nc.alloc_sbuf_tensor("sb_sb25", [MD, 2], F32).ap()
    flatu = nc.alloc_sbuf_tensor("sb_flatu", [MD, 1], U32).ap()
    boxu = nc.alloc_sbuf_tensor("sb_boxu", [MD, 1], U32).ap()
    cxy = nc.alloc_sbuf_tensor("sb_cxy", [MD, 4], F32).ap()
    mrow = nc.alloc_sbuf_tensor("sb_mrow", [MD, 32], F32).ap()

    # pools for big transients
    pool = ctx.enter_context(tc.tile_pool(name="big", bufs=1))
    ppool = ctx.enter_context(tc.tile_pool(name="pr", bufs=2))
    psum = ctx.enter_context(
        tc.tile_pool(name="ps", bufs=2, space=bass.MemorySpace.PSUM))
    psum2 = ctx.enter_context(
        tc.tile_pool(name="ps2", bufs=1, space=bass.MemorySpace.PSUM))

    # stage small consts into SBUF
    const_shapes = {"tokbase": (NCAND_P, 1), "ones": (1, 128),
                    "ident": (128, 128), "offcol": (128, 1),
                    "biascol": (128, 1), "hgrid": (MD, 160), "yneg": (80, 1),
                    "selbc": (MD, MD * 80), "slotgrid": (128, 27)}
    cst = {}
    for name, shp in const_shapes.items():
        t = nc.alloc_sbuf_tensor("sbc_" + name, list(shp), F32).ap()
        nc.sync.dma_start(t, ins[name])
        cst[name] = t
    ins = {**ins, **cst}

    scores = ins["scores_b"]
    # ------------- 1. topk per token ------------------------------------
    nc.vector.memset(sc2[:, :], NEG)
    nc.sync.dma_start(sc1[:, :],
                      scores[0:128 * 3840].rearrange("(p f) -> p f", p=128))
    nc.sync.dma_start(sc2[0:32, :],
                      scores[8 * VOCAB:8 * VOCAB + 32 * 3840]
                      .rearrange("(p f) -> p f", p=32))
    nc.sync.dma_start(sc2[32:47, :],
                      scores[10 * VOCAB:10 * VOCAB + 15 * 3840]
                      .rearrange("(p f) -> p f", p=15))
    with tc.tile_critical():
        nc.gpsimd.load_library(library_config.topk)
        nc.gpsimd.topk(tk1[:], sc1[:], tokens=8, vocab_size=VOCAB, k=256)
        nc.gpsimd.topk(tk2[:], sc2[:], tokens=3, vocab_size=VOCAB, k=256)

    # ------------- 2. candidates -> [1, CAP] rows -----------------------
    # top-32 of token t = partitions 16t+14, 16t+15 (ascending vals cols
    # 0..15, idx cols 16..31)
    nc.vector.memset(vflat, NEG)
    nc.vector.memset(fflat, -1.0)
    cv = nc.alloc_sbuf_tensor("sb_cv", [NCAND_P, 16], F32).ap()
    cfi = nc.alloc_sbuf_tensor("sb_cfi", [NCAND_P, 16], U32).ap()
    cff = nc.alloc_sbuf_tensor("sb_cff", [NCAND_P, 16], F32).ap()
    for t in range(11):
        srcv = tk1 if t < 8 else tk2
        pb = 16 * (t if t < 8 else t - 8)
        nc.sync.dma_start(cv[2 * t:2 * t + 2, :],
                          srcv[pb + 14:pb + 16, 0:16].bitcast(F32))
        nc.sync.dma_start(cfi[2 * t:2 * t + 2, :],
                          srcv[pb + 14:pb + 16, 16:32])
    nc.vector.tensor_copy(cff, cfi)  # exact ints
    nc.vector.tensor_scalar(cff, cff, ins_tokbase(nc, ins), None, op0=A.add)
    # resort to [1, 352]: r = p*16 + s
    nc.sync.dma_start(vflat[:, 0:NCAND_P * 16], cv)
    nc.sync.dma_start(fflat[:, 0:NCAND_P * 16], cff)
    # cols: vcol[p,c] = vflat[3p+c] (lockstep DMA iteration)
    nc.sync.dma_start(vcol, vflat)
    nc.sync.dma_start(fcol, fflat)
    # V_row/F_row = ones-matmul broadcasts (PSUM)
    vrow = psum2.tile([128, CAP], F32)
    frow = psum2.tile([128, CAP], F32)
    nc.tensor.matmul(vrow[:], ins["ones"], vflat, start=True, stop=True)
    nc.tensor.matmul(frow[:], ins["ones"], fflat, start=True, stop=True)
    # ------------- ranks ------------------------------------------------
    for c in range(3):
        nc.vector.tensor_scalar(acc, vrow[:], vcol[:, c:c + 1], None,
                                op0=A.is_equal)
        nc.vector.scalar_tensor_tensor(acc, frow[:], fcol[:, c:c + 1], acc,
                                       op0=A.is_lt, op1=A.mult)
        nc.vector.scalar_tensor_tensor(acc, vrow[:], vcol[:, c:c + 1], acc,
                                       op0=A.is_gt, op1=A.add,
                                       accum_out=rank[:, c:c + 1])
    # ------------- targets + scatter ------------------------------------
    tgtf = nc.alloc_sbuf_tensor("sb_tgtf", [128, 3], F32).ap()
    nc.vector.tensor_scalar(tgtf, rank, ins["offcol"], None, op0=A.subtract)
    neg26 = nc.alloc_sbuf_tensor("sb_neg26", [128, 3], F32).ap()
    nc.vector.memset(neg26, 26.0)
    ltz = nc.alloc_sbuf_tensor("sb_ltz", [128, 3], mybir.dt.uint8).ap()
    lzf = nc.alloc_sbuf_tensor("sb_lzf", [128, 3], F32).ap()
    nc.vector.tensor_scalar(lzf, tgtf, 0.0, None, op0=A.is_lt)
    nc.vector.tensor_copy(ltz, lzf)
    nc.vector.tensor_scalar(tgtf, tgtf, 26.0, None, op0=A.min)
    nc.vector.copy_predicated(tgtf, ltz, neg26)
    nc.scalar.activation(sigc, vcol, mybir.ActivationFunctionType.Sigmoid)
    # pack [sig, flat]
    nc.vector.tensor_copy(packed[:, :, 0:1], sigc.rearrange("p (c x) -> p c x", x=1))
    nc.vector.tensor_copy(packed[:, :, 1:2], fcol.rearrange("p (c x) -> p c x", x=1))
    # de-scatter via permutation matmul: sorted[slot,:] = sum_r
    # 1[tgt_r == slot] * packed[r,:]   (slot grid is a host const [128,27])
    pmat = nc.alloc_sbuf_tensor("sb_pmat", [128, 27], F32).ap()
    srt_ps = psum2.tile([27, 2], F32, tag="bc")
    for c in range(3):
        nc.vector.tensor_scalar(pmat, ins["slotgrid"], tgtf[:, c:c + 1], None,
                                op0=A.is_equal)
        nc.tensor.matmul(srt_ps[:], pmat, packed[:, c, :],
                         start=(c == 0), stop=(c == 2))
    nc.vector.tensor_copy(sb25, srt_ps[0:MD, :])
    nc.vector.tensor_copy(flatu, sb25[:, 1:2])
    nc.gpsimd.indirect_dma_start(
        out=boxu,
        out_offset=None,
        in_=ins["boxlut"],
        in_offset=IndirectOffsetOnAxis(ap=flatu, axis=0),
        bounds_check=671999, oob_is_err=False)
    nc.gpsimd.indirect_dma_start(
        out=cxy,
        out_offset=None, in_=ins["boxes_b"],
        in_offset=IndirectOffsetOnAxis(ap=boxu, axis=0),
        bounds_check=8399, oob_is_err=False)
    nc.gpsimd.indirect_dma_start(
        out=mrow,
        out_offset=None, in_=ins["masks_b"],
        in_offset=IndirectOffsetOnAxis(ap=boxu, axis=0),
        bounds_check=8399, oob_is_err=False)
    # label = flat - 80*box (int)
    lab_i = nc.alloc_sbuf_tensor("sb_lab_i", [MD, 1], I32).ap()
    box_i = nc.alloc_sbuf_tensor("sb_box_i", [MD, 1], I32).ap()
    fl_i = nc.alloc_sbuf_tensor("sb_fl_i", [MD, 1], I32).ap()
    nc.vector.tensor_copy(box_i, boxu)
    nc.vector.tensor_copy(fl_i, flatu)
    nc.vector.tensor_scalar(box_i, box_i, 80, None, op0=A.mult)
    nc.vector.tensor_tensor(lab_i, fl_i, box_i, op=A.subtract)
    labf = nc.alloc_sbuf_tensor("sb_labf", [MD, 1], F32).ap()
    nc.vector.tensor_copy(labf, lab_i)
    # masksel transpose [25,32] -> [32,25]
    mT_ps = psum2.tile([32, MD], F32, tag="bc")
    nc.tensor.transpose(out=mT_ps[:], in_=mrow, identity=ins["ident"][0:MD, 0:MD])
    mT = nc.alloc_sbuf_tensor("sb_mT", [32, MD], F32).ap()
    nc.vector.tensor_copy(mT, mT_ps[:])
    # ------------- geometry ---------------------------------------------
    # proto coords: x1p=(cx-w/2)*160 etc; bins: binw=(x2p-x1p)/160 = w*1.0
    g = nc.alloc_sbuf_tensor("sb_g", [MD, 8], F32).ap()
    cx, cy, w, h = (cxy[:, i:i + 1] for i in range(4))
    x1p, y1p, x2p, y2p = (g[:, i:i + 1] for i in range(4))
    binw, binh = g[:, 4:5], g[:, 5:6]
    nc.vector.scalar_tensor_tensor(x1p, w, -0.5, cx, op0=A.mult, op1=A.add)
    nc.vector.scalar_tensor_tensor(y1p, h, -0.5, cy, op0=A.mult, op1=A.add)
    nc.vector.scalar_tensor_tensor(x2p, w, 0.5, cx, op0=A.mult, op1=A.add)
    nc.vector.scalar_tensor_tensor(y2p, h, 0.5, cy, op0=A.mult, op1=A.add)
    for col in (x1p, y1p, x2p, y2p):
        nc.vector.tensor_scalar(col, col, 160.0, None, op0=A.mult)
    nc.vector.tensor_scalar(binw, w, 1.0, None, op0=A.mult)  # w*640*0.25/160
    nc.vector.tensor_scalar(binh, h, 1.0, None, op0=A.mult)
    # grids ys2 [25, 320] = [clip(ys) || clip(ys)-80], xs2 likewise
    ys2 = nc.alloc_sbuf_tensor("sb_ys2", [MD, 320], F32).ap()
    xs2 = nc.alloc_sbuf_tensor("sb_xs2", [MD, 320], F32).ap()
    for gi, (dst, b0, c0) in enumerate(((ys2, binh, y1p), (xs2, binw, x1p))):
        off05 = nc.alloc_sbuf_tensor(f"sb_off05_{gi}", [MD, 1], F32).ap()
        nc.vector.tensor_scalar(off05, c0, 0.5, None, op0=A.subtract)
        nc.vector.tensor_scalar(dst[:, 0:160], ins["hgrid"], b0, off05,
                                op0=A.mult, op1=A.add)
        nc.vector.tensor_scalar(dst[:, 0:160], dst[:, 0:160], 0.0, 159.0,
                                op0=A.max, op1=A.min)
        nc.vector.tensor_scalar(dst[:, 160:320], dst[:, 0:160], 80.0, None,
                                op0=A.subtract)
    # rows [1, 8000] (order t, yc, h)

    # ------------- hat matrices  [80, 8000] -----------------------------
    # hatY[y', (t,yc,h)] = -(min(|ys2row - y'|,1) - 1) sign-pair cancels
    def build_hat(hat, row):
        for ch in range(MD):
            sl = slice(320 * ch, 320 * (ch + 1))
            bc = psum2.tile([80, 320], F32, tag="bc")
            nc.tensor.matmul(bc[:], ins["selbc"][:, 80 * ch:80 * ch + 80],
                             row, start=True, stop=True)
            nc.scalar.activation(hat[:, sl], bc[:],
                                 mybir.ActivationFunctionType.Abs,
                                 bias=ins["yneg"])
        # pass2: -hat = min(a,1) - 1  (fused)
        for half in range(2):
            sl = slice(4000 * half, 4000 * (half + 1))
            nc.vector.tensor_scalar(hat[:, sl], hat[:, sl], 1.0, 1.0,
                                    op0=A.min, op1=A.subtract)

    hatY = pool.tile([80, 8000], F32, tag="hat")
    build_hat(hatY, ys2)
    # ------------- M matmul + eviction + reshape ------------------------
    mdram = nc.dram_tensor("mdram", [MD, 160, 160], F32).ap()
    mdflat = mdram.rearrange("t y x -> t (y x)")
    for cg in range(9):
        base = cg * 2880
        nn_ = min(2880, 25600 - base)
        pch = ppool.tile([32, 2880], F32, tag="pch")
        nc.sync.dma_start(pch[:, 0:nn_], ins["protos_b"][:, base:base + nn_])
        for ci in range((nn_ + 479) // 480):
            c = cg * 6 + ci
            n = min(480, nn_ - 480 * ci)
            mp_ps = psum.tile([MD, 480], F32, tag="mps")
            nc.tensor.matmul(mp_ps[:, 0:n], mT, pch[:, 480 * ci:480 * ci + n],
                             start=True, stop=True)
            mchunk = ppool.tile([MD, 480], F32, tag="mchunk")
            if c % 2 == 0:
                nc.vector.tensor_copy(mchunk[:, 0:n], mp_ps[:, 0:n])
            else:
                nc.scalar.copy(mchunk[:, 0:n], mp_ps[:, 0:n])
            nc.sync.dma_start(mdflat[:, 480 * c:480 * c + n], mchunk[:, 0:n])
    mxy = pool.tile([80, MD, 2, 160], F32, tag="mxy")
    nc.sync.dma_start(
        mxy[:], mdram.rearrange("t (yc y) x -> y t yc x", y=80))
    # ------------- stage A: tmpT = M^T @ GyhatT -------------------------
    tx = pool.tile([80, MD, 2, 160], F32, tag="tx")
    for t in range(MD):
        for xc in range(2):
            pa = psum.tile([80, 160], F32, tag="ab")
            for yc in range(2):
                nc.tensor.matmul(pa[:], mxy[:, t, yc, 80 * xc:80 * xc + 80],
                                 hatY[:, (t * 2 + yc) * 160:(t * 2 + yc) * 160 + 160],
                                 start=(yc == 0), stop=(yc == 1))
            if (t + xc) % 2 == 0:
                nc.vector.tensor_copy(tx[:, t, xc, :], pa[:])
            else:
                nc.scalar.copy(tx[:, t, xc, :], pa[:])
    # ------------- stage B: mp = tmpT^T @ GxhatT + bias -----------------
    hatX = pool.tile([80, 8000], F32, tag="hat")
    build_hat(hatX, xs2)
    mph0 = pool.tile([128, MD, 160], F32, tag="mph0")
    mph1 = pool.tile([32, MD, 160], F32, tag="mph1")
    for t in range(MD):
        for hc in range(2):
            hn = 128 if hc == 0 else 32
            pb = psum.tile([hn, 160], F32, tag="ab")
            for xc in range(2):
                nc.tensor.matmul(pb[:], tx[:, t, xc, 128 * hc:128 * hc + hn],
                                 hatX[:, (t * 2 + xc) * 160:(t * 2 + xc) * 160 + 160],
                                 start=(xc == 0), stop=(xc == 1))
            dst = mph0[:, t, :] if hc == 0 else mph1[:, t, :]
            nc.scalar.activation(dst, pb[:],
                                 mybir.ActivationFunctionType.Identity,
                                 bias=ins["biascol"][0:hn, :])
    # ------------- output writes ----------------------------------------
    ob = outs["out_block"]
    nc.sync.dma_start(
        ob[:, 6:6 + 128 * 160].rearrange("t (y x) -> y t x", y=128), mph0[:])
    nc.sync.dma_start(
        ob[:, 6 + 128 * 160:6 + 160 * 160].rearrange("t (y x) -> y t x", y=32),
        mph1[:])
    if "dbg_vflat" in outs:
        nc.sync.dma_start(outs["dbg_vflat"], vflat)
        nc.sync.dma_start(outs["dbg_fflat"], fflat)
        nc.sync.dma_start(outs["dbg_rank"], rank)
        nc.sync.dma_start(outs["dbg_sb25"], sb25)
        nc.sync.dma_start(outs["dbg_cxy"], cxy)
        nc.sync.dma_start(outs["dbg_vcol"], vcol)
        nc.sync.dma_start(outs["dbg_fcol"], fcol)
        nc.sync.dma_start(outs["dbg_tgt"], tgtf)
        nc.sync.dma_start(outs["dbg_packed"], packed.rearrange("p a b -> p (a b)"))
    asm = nc.alloc_sbuf_tensor("sb_asm", [MD, 6], F32).ap()
    for i, col in enumerate((x1p, y1p, x2p, y2p)):
        nc.vector.tensor_scalar(asm[:, i:i + 1], col, 4.0, None, op0=A.mult)
    nc.vector.tensor_copy(asm[:, 4:5], sb25[:, 0:1])
    nc.vector.tensor_copy(asm[:, 5:6], labf)
    nc.sync.dma_start(ob[:, 0:6], asm)
    ctx.close()


def ins_tokbase(nc, ins):
    return ins["tokbase"]


_BUILD_CACHE = {}


def build():
    if "nc" in _BUILD_CACHE:
        return _BUILD_CACHE["nc"]
    nc = bacc.Bacc("TRN2", target_bir_lowering=False, debug=False, num_devices=8)
    ins, outs = {}, {}

    def inp(name, shape, dt):
        ins[name] = nc.dram_tensor(name, list(shape), dt, kind="ExternalInput").ap()

    inp("scores_b", (672000,), F32)
    inp("protos_b", (32, 25600), F32)
    inp("masks_b", (8400, 32), F32)
    inp("boxes_b", (8400, 4), F32)
    inp("boxlut", (672000, 1), U32)
    inp("tokbase", (NCAND_P, 1), F32)
    inp("ones", (1, 128), F32)
    inp("ident", (128, 128), F32)
    inp("offcol", (128, 1), F32)
    inp("biascol", (128, 1), F32)
    inp("hgrid", (MD, 160), F32)
    inp("yneg", (80, 1), F32)
    inp("selbc", (MD, MD * 80), F32)
    inp("slotgrid", (128, 27), F32)
    outs["out_block"] = nc.dram_tensor("out_block", [MD, OUTW], F32,
                                       kind="ExternalOutput").ap()
    import os
    if os.environ.get("KDBG") == "1":
        for nm, shp in (("dbg_vflat", (1, CAP)), ("dbg_fflat", (1, CAP)),
                        ("dbg_rank", (128, 3)), ("dbg_sb25", (MD, 2)),
                        ("dbg_cxy", (MD, 4)), ("dbg_vcol", (128, 3)),
                        ("dbg_fcol", (128, 3)), ("dbg_tgt", (128, 3)),
                        ("dbg_packed", (128, 6))):
            outs[nm] = nc.dram_tensor(nm, list(shp), F32,
                                      kind="ExternalOutput").ap()
    with tile.TileContext(nc) as tc:
        emit_kernel(tc, ins, outs)
    nc.compile()
    _BUILD_CACHE["nc"] = nc
    return nc


def make_in_maps(boxes, scores, protos, masks, mask_bias):
    B = scores.shape[0]
    boxlut = (np.arange(672000, dtype=np.uint32) // 80).astype(np.uint32)[:, None]
    tokbase = np.repeat(np.arange(11, dtype=np.float32) * VOCAB, 2)[:, None]
    ones = np.ones((1, 128), np.float32)
    ident = np.eye(128, dtype=np.float32)
    hgrid = np.tile(np.arange(160, dtype=np.float32) + 0.5, (MD, 1))
    yneg = -np.arange(80, dtype=np.float32)[:, None]
    selbc = np.zeros((MD, MD * 80), np.float32)
    for t in range(MD):
        selbc[t, 80 * t:80 * t + 80] = 1.0
    in_maps = []
    for core in range(8):
        b, q = core // 4, core % 4
        in_maps.append({
            "scores_b": np.ascontiguousarray(scores[b].reshape(-1)),
            "protos_b": np.ascontiguousarray(protos[b].reshape(32, 25600)),
            "masks_b": np.ascontiguousarray(masks[b]),
            "boxes_b": np.ascontiguousarray(boxes[b]),
            "boxlut": boxlut,
            "tokbase": tokbase.astype(np.float32),
            "ones": ones, "ident": ident,
            "offcol": np.full((128, 1), 25.0 * q, np.float32),
            "biascol": np.full((128, 1), float(mask_bias[0]), np.float32),
            "hgrid": hgrid, "yneg": yneg, "selbc": selbc,
            "slotgrid": np.tile(np.arange(27, dtype=np.float32), (128, 1)),
        })
    return in_maps


def kernel(boxes, scores, protos, masks, mask_bias):
    nc = build()
    in_maps = make_in_maps(boxes, scores, protos, masks, mask_bias)
    res = run_bass_kernel_spmd(nc, in_maps, list(range(8)))
    out = np.zeros((2, 100, OUTW), np.float32)
    for core in range(8):
        b, q = core // 4, core % 4
        out[b, 25 * q:25 * q + 25] = res.results[core]["out_block"]
    return out
